# revision 1
# baseline (speedup 1.0000x reference)
"""GAT 2-layer network on 8 Trainium2 NeuronCores.

Strategy (edge-parallel, per the sharding hint "partition edges, replicate
node features"):
  - Nodes are sorted by in-degree and packed into 128-node tiles; tiles are
    dealt round-robin onto the 8 cores so every core runs the identical
    instruction stream (SPMD) over a shared per-step K schedule.
  - All FLOPs run on device across 3 launches:
      K1: xp1 = x @ W1 plus per-head attention dot products (s1, ad1).
      K2: per dst-tile segment softmax + message aggregation for layer 1,
          ELU, then xp2 = h @ W2ext (fused) -> layer-2 node table.
      K3: layer-2 segment softmax + aggregation + bias + log_softmax.
  - Between launches the host only does index-based data movement: it
    replicates the device-computed per-node tables into per-edge-slot
    streams (degree-padded, p-major) so each device step reads purely
    sequential DMA. No floating-point math happens on the host.
"""

import os
import sys

for _p in ("/opt/trn_rl_repo", "/root/.axon_site/_ro/trn_rl_repo"):
    if os.path.isdir(_p) and _p not in sys.path:
        sys.path.insert(0, _p)

import numpy as np

import concourse.bacc as bacc
import concourse.bass as bass
import concourse.tile as tile
from concourse import mybir
from concourse.bass_utils import run_bass_kernel_spmd

F32 = mybir.dt.float32
AF = mybir.ActivationFunctionType
ALU = mybir.AluOpType
AX = mybir.AxisListType

N = 100000
E = 1600000
F_IN = 256
H1, D1 = 8, 8
HD1 = H1 * D1          # 64
D2 = 16                # H2 = 1
NEG = 0.2
NC = 8
P = 128
TILES = 784            # ceil(100000 / 128) rounded up to a multiple of 8
STEPS = TILES // NC    # 98
NPC = STEPS * P        # 12544 node rows handled per core in K1
PADS = -1.0e38         # sentinel: exp(lrelu(PADS + ad)) == 0 exactly

TRACE = False          # test.py flips this for NTFF profiling
SIM = False            # run through CoreSim instead of hardware
SIM_CORES = None       # e.g. [0] to only simulate core 0
LAST_EXEC_NS = []      # per-launch exec_time_ns when TRACE


def _run(nc, in_maps, tag):
    if SIM:
        from concourse.bass_interp import CoreSim

        outs = []
        cores = range(NC) if SIM_CORES is None else SIM_CORES
        for c in range(NC):
            if c not in cores:
                outs.append(outs[-1] if outs else {})
                continue
            sim = CoreSim(nc, trace=False)
            for k, v in in_maps[c].items():
                sim.tensor(k)[:] = v
            sim.simulate(check_with_hw=False)
            onames = [
                a.memorylocations[0].name
                for a in nc.m.functions[0].allocations
                if isinstance(a, mybir.MemoryLocationSet) and a.kind == "ExternalOutput"
            ]
            outs.append({k: np.array(sim.tensor(k)) for k in onames})
        return outs
    if TRACE:
        import hookfix  # noqa: F401  (registers antenv.axon_hooks)

        hookfix.install()
    res = run_bass_kernel_spmd(nc, in_maps, list(range(NC)), trace=TRACE)
    if TRACE:
        LAST_EXEC_NS.append((tag, res.exec_time_ns))
    return res.results


def _bc(ap, shape):
    """Broadcast the free dims of `ap` to `shape` (partition dim must already
    match).  Target dims are matched against source free dims right-to-left;
    size-1 source dims and unmatched target dims become step-0 (broadcast)."""
    src = ap.ap
    assert src[0][1] == shape[0], (src, shape)
    sdims = list(src[1:])
    res = []
    si = len(sdims) - 1
    for ti in range(len(shape) - 1, 0, -1):
        if si >= 0 and sdims[si][1] == shape[ti]:
            res.append(sdims[si])
            si -= 1
        elif si >= 0 and sdims[si][1] == 1:
            res.append([0, shape[ti]])
            si -= 1
        else:
            res.append([0, shape[ti]])
    assert si < 0, (src, shape)
    return bass.AP(tensor=ap.tensor, offset=ap.offset, ap=[src[0]] + res[::-1])


def _tail0(ap, n):
    """Append a trailing step-0 (broadcast) dim of size n."""
    return bass.AP(tensor=ap.tensor, offset=ap.offset, ap=list(ap.ap) + [[0, n]])


def _mid0(ap, pos, n):
    """Insert a step-0 (broadcast) dim of size n at free-dim position pos
    (ap.ap index pos, counting the partition dim as 0)."""
    dims = list(ap.ap)
    return bass.AP(
        tensor=ap.tensor, offset=ap.offset, ap=dims[:pos] + [[0, n]] + dims[pos:]
    )


def _rep_row(nc, pool, dram_t, nparts, cols, tag):
    """DMA-replicate a flat `cols`-element DRAM tensor across `nparts`
    partitions (engines cannot broadcast across partitions themselves)."""
    tl = pool.tile([nparts, cols], F32, tag=tag)
    src = bass.AP(tensor=dram_t[:].tensor, offset=0, ap=[[0, nparts], [1, cols]])
    nc.sync.dma_start(tl[:], src)
    return tl


# --------------------------------------------------------------------------
# K1: node tables.  out column-major xq1T [80, NPC] per core:
#     rows 0:64 xp1 = x @ W1, 64:72 s1 (att_src dot), 72:80 ad1 (att_dst dot)
# --------------------------------------------------------------------------
def build_k1():
    nc = bacc.Bacc("TRN2", target_bir_lowering=False, debug=False, num_devices=NC)
    xT = nc.dram_tensor("xT", [F_IN, NPC], F32, kind="ExternalInput")
    w1 = nc.dram_tensor("w1", [F_IN, HD1], F32, kind="ExternalInput")
    as1 = nc.dram_tensor("as1", [H1, D1], F32, kind="ExternalInput")
    ad1 = nc.dram_tensor("ad1", [H1, D1], F32, kind="ExternalInput")
    out = nc.dram_tensor("xq1T", [80, NPC], F32, kind="ExternalOutput")

    with tile.TileContext(nc) as tc:
        with (
            tc.tile_pool(name="pro", bufs=1) as pro,
            tc.tile_pool(name="io", bufs=3) as io,
            tc.tile_pool(name="ps", bufs=4, space="PSUM") as ps,
        ):
            w1sb = pro.tile([P, 2, HD1], F32)
            nc.sync.dma_start(w1sb[:], w1[:].rearrange("(c p) d -> p c d", p=P))
            asr = _rep_row(nc, pro, as1, P, HD1, "asr")
            adr = _rep_row(nc, pro, ad1, P, HD1, "adr")

            # w_s1[f, h] = sum_d W1[f, h*8+d] * att_src1[h, d]; same for dst
            wext = pro.tile([P, 2, 80], F32)
            nc.scalar.copy(wext[:, :, 0:HD1], w1sb[:])
            for att, lo in ((asr, 64), (adr, 72)):
                tmp = pro.tile([P, 2, HD1], F32, tag="k1tmp")
                nc.vector.tensor_tensor(
                    tmp[:], w1sb[:], _bc(att[:], [P, 2, HD1]), op=ALU.mult
                )
                nc.vector.tensor_reduce(
                    wext[:, :, lo : lo + 8],
                    tmp[:].rearrange("p c (h d) -> p c h d", d=D1),
                    axis=AX.X,
                    op=ALU.add,
                )

            xTr = xT[:].rearrange("(c p) n -> p c n", p=P)
            GT = 4                                  # node-tiles per matmul
            for t0 in range(0, STEPS, GT):
                g = min(GT, STEPS - t0)
                W = g * P
                xt = io.tile([P, 2, GT * P], F32, tag="xt")
                nc.sync.dma_start(xt[:, :, 0:W], xTr[:, :, t0 * P : t0 * P + W])
                pt = ps.tile([80, GT * P], F32, tag="k1ps")
                nc.tensor.matmul(
                    pt[:, 0:W], lhsT=wext[:, 0, :], rhs=xt[:, 0, 0:W],
                    start=True, stop=False,
                )
                nc.tensor.matmul(
                    pt[:, 0:W], lhsT=wext[:, 1, :], rhs=xt[:, 1, 0:W],
                    start=False, stop=True,
                )
                ot = io.tile([80, GT * P], F32, tag="k1o")
                nc.vector.tensor_copy(ot[:, 0:W], pt[:, 0:W])
                nc.sync.dma_start(out[:, t0 * P : t0 * P + W], ot[:, 0:W])
    nc.compile()
    return nc


# --------------------------------------------------------------------------
# K2: layer-1 edge aggregation + ELU + fused xp2/s2/ad2 table.
#   EV1 row (72 f32): [xp1(64) | s1(8)] for the slot's src node (PADS rows
#   have s1 = -1e38 so exp()==0).  p-major slots: slot = base + p*K + k.
#   out t2T [18, NPC] column-major: rows 0:16 xp2, 16 s2, 17 ad2.
# --------------------------------------------------------------------------
def build_k2(groups, k_tile):
    slots = P * sum(g * kb for _, g, kb in groups)
    nc = bacc.Bacc("TRN2", target_bir_lowering=False, debug=False, num_devices=NC)
    evs = nc.dram_tensor("ev1s", [8 * slots], F32, kind="ExternalInput")
    evx = nc.dram_tensor("ev1x", [64 * slots], F32, kind="ExternalInput")
    adt = nc.dram_tensor("adR", [NPC, H1], F32, kind="ExternalInput")
    w2 = nc.dram_tensor("w2", [HD1, D2], F32, kind="ExternalInput")
    as2 = nc.dram_tensor("as2", [1, D2], F32, kind="ExternalInput")
    ad2 = nc.dram_tensor("ad2", [1, D2], F32, kind="ExternalInput")
    b1t = nc.dram_tensor("b1", [HD1], F32, kind="ExternalInput")
    out = nc.dram_tensor("t2T", [18, NPC], F32, kind="ExternalOutput")

    from concourse.masks import make_identity

    with tile.TileContext(nc) as tc:
        with (
            tc.tile_pool(name="pro", bufs=1) as pro,
            tc.tile_pool(name="io", bufs=2) as io,
            tc.tile_pool(name="wk", bufs=2) as wk,
            tc.tile_pool(name="ps", bufs=2, space="PSUM") as ps,
        ):
            w2sb = pro.tile([HD1, D2], F32)
            nc.sync.dma_start(w2sb[:], w2[:])
            a2s = _rep_row(nc, pro, as2, HD1, D2, "a2s")
            a2d = _rep_row(nc, pro, ad2, HD1, D2, "a2d")
            b1r = _rep_row(nc, pro, b1t, P, HD1, "b1r")
            ident = pro.tile([P, P], F32)
            make_identity(nc, ident[:])
            c_eps = pro.tile([P, 1], F32)
            nc.vector.memset(c_eps[:], 1e-16)
            c_m1 = pro.tile([P, 1], F32)
            nc.vector.memset(c_m1[:], -1.0)

            # W2ext [64, 18] = [W2 | W2@att_src2 | W2@att_dst2]
            w2e = pro.tile([HD1, 18], F32)
            nc.scalar.copy(w2e[:, 0:D2], w2sb[:])
            for att, col in ((a2s, 16), (a2d, 17)):
                tmp2 = pro.tile([HD1, D2], F32, tag="k2tmp")
                nc.vector.tensor_tensor(tmp2[:], w2sb[:], att[:], op=ALU.mult)
                nc.vector.tensor_reduce(
                    w2e[:, col : col + 1], tmp2[:], axis=AX.X, op=ALU.add
                )

            base = 0
            for t0, G, K in groups:
                est = io.tile([P, G, 8 * K], F32, tag="evs")
                nc.sync.dma_start(
                    est[:],
                    evs[8 * base : 8 * (base + P * G * K)].rearrange(
                        "(p g f) -> p g f", g=G, f=8 * K
                    ),
                )
                ext = io.tile([P, G, 64 * K], F32, tag="evx")
                nc.sync.dma_start(
                    ext[:],
                    evx[64 * base : 64 * (base + P * G * K)].rearrange(
                        "(p g f) -> p g f", g=G, f=64 * K
                    ),
                )
                base += P * G * K
                adv = io.tile([P, G, H1], F32, tag="ad")
                nc.sync.dma_start(
                    adv[:],
                    adt[t0 * P : (t0 + G) * P, :].rearrange("(g p) h -> p g h", p=P),
                )

                # ex = exp(lrelu(s1+ad1)) = exp(0.2 e) * exp(relu(0.8 e))
                e = wk.tile([P, G, H1, K], F32, tag="e")
                nc.vector.tensor_tensor(
                    e[:],
                    est[:].rearrange("p g (h k) -> p g h k", k=K),
                    _tail0(adv[:], K),
                    op=ALU.add,
                )
                ea = wk.tile([P, G, H1, K], F32, tag="ea")
                nc.scalar.activation(ea[:], e[:], AF.Exp, scale=NEG)
                eb = wk.tile([P, G, H1, K], F32, tag="eb")
                nc.scalar.activation(eb[:], e[:], AF.Relu, scale=1.0 - NEG)
                nc.scalar.activation(eb[:], eb[:], AF.Exp)
                ex = ea
                nc.vector.tensor_tensor(ex[:], ea[:], eb[:], op=ALU.mult)

                # denom + reciprocal
                dn = wk.tile([P, G, H1], F32, tag="dn")
                nc.vector.tensor_reduce(dn[:], ex[:], axis=AX.X, op=ALU.add)
                inv = wk.tile([P, G, H1], F32, tag="inv")
                nc.scalar.activation(inv[:], dn[:], AF.Identity, bias=c_eps[:])
                nc.vector.reciprocal(inv[:], inv[:])

                # msg[p,(g h),d,k] = ex * xp — one TT per group, with the k
                # axis split between gpsimd (bulk) and DVE (remainder).
                msg = wk.tile([P, G * H1, D1, K], F32, tag="msg")
                xpall = ext[:].rearrange("p g (h d k) -> p (g h) d k", d=D1, k=K)
                exall = ex[:].rearrange("p g h k -> p (g h) k")
                nc.vector.tensor_tensor(
                    msg[:], xpall[:], _mid0(exall[:], 2, D1), op=ALU.mult
                )
                agg = wk.tile([P, G, H1, D1], F32, tag="agg")
                nc.vector.tensor_reduce(
                    agg[:].rearrange("p g h d -> p (g h) d"),
                    msg[:],
                    axis=AX.X,
                    op=ALU.add,
                )

                # h = elu(agg * inv + b1)
                hsb = wk.tile([P, G, HD1], F32, tag="hsb")
                nc.vector.tensor_tensor(
                    hsb[:].rearrange("p g (h d) -> p g h d", d=D1),
                    agg[:],
                    _tail0(inv[:], D1),
                    op=ALU.mult,
                )
                nc.vector.tensor_tensor(
                    hsb[:], hsb[:], _bc(b1r[:], [P, G, HD1]), op=ALU.add
                )
                hpos = wk.tile([P, G, HD1], F32, tag="hpos")
                nc.scalar.activation(hpos[:], hsb[:], AF.Relu)
                nc.vector.tensor_tensor(hsb[:], hsb[:], hpos[:], op=ALU.subtract)
                nc.scalar.activation(hsb[:], hsb[:], AF.Exp)  # exp(min(h,0))
                nc.vector.tensor_tensor(hsb[:], hsb[:], hpos[:], op=ALU.add)
                nc.scalar.activation(hsb[:], hsb[:], AF.Identity, bias=c_m1[:])

                # xp2/s2/ad2 via per-tile transpose + matmul
                shT = wk.tile([HD1, G, P], F32, tag="shT")
                pt2 = ps.tile([18, G, P], F32, tag="pt2")
                for g in range(G):
                    phT = ps.tile([HD1, P], F32, tag="phT")
                    nc.tensor.transpose(phT[:], hsb[:, g, :], ident[:])
                    nc.scalar.copy(shT[:, g, :], phT[:])
                    nc.tensor.matmul(
                        pt2[:, g, :], lhsT=w2e[:], rhs=shT[:, g, :],
                        start=True, stop=True,
                    )
                st2 = io.tile([18, G, P], F32, tag="st2")
                nc.scalar.copy(st2[:], pt2[:])
                nc.sync.dma_start(
                    out[:, t0 * P : (t0 + G) * P],
                    st2[:].rearrange("r g n -> r (g n)"),
                )
    nc.compile()
    return nc


# --------------------------------------------------------------------------
# K3: layer-2 edge aggregation + bias + log_softmax.
#   EV2 row (18 f32): [xp2(16) | s2(1) | pad] for the slot's src node.
# --------------------------------------------------------------------------
def build_k3(groups):
    tot = 17 * P * sum(g * kb for _, g, kb in groups)
    nc = bacc.Bacc("TRN2", target_bir_lowering=False, debug=False, num_devices=NC)
    ev = nc.dram_tensor("ev2", [tot], F32, kind="ExternalInput")
    adt = nc.dram_tensor("ad2R", [NPC, 1], F32, kind="ExternalInput")
    b2t = nc.dram_tensor("b2", [D2], F32, kind="ExternalInput")
    out = nc.dram_tensor("o3", [NPC, D2], F32, kind="ExternalOutput")

    with tile.TileContext(nc) as tc:
        with (
            tc.tile_pool(name="pro", bufs=1) as pro,
            tc.tile_pool(name="io", bufs=3) as io,
            tc.tile_pool(name="wk", bufs=2) as wk,
        ):
            b2r = _rep_row(nc, pro, b2t, P, D2, "b2r")

            base = 0
            for t0, G, K in groups:
                evt = io.tile([P, G, 17 * K], F32, tag="ev")
                nc.sync.dma_start(
                    evt[:],
                    ev[base : base + P * G * 17 * K].rearrange(
                        "(p g f) -> p g f", g=G, f=17 * K
                    ),
                )
                base += P * G * 17 * K
                adv = io.tile([P, G, 1], F32, tag="ad")
                nc.sync.dma_start(
                    adv[:],
                    adt[t0 * P : (t0 + G) * P, :].rearrange("(g p) o -> p g o", p=P),
                )

                e = wk.tile([P, G, K], F32, tag="e")
                nc.vector.tensor_tensor(
                    e[:], evt[:, :, 16 * K : 17 * K], _bc(adv[:], [P, G, K]), op=ALU.add
                )
                et = wk.tile([P, G, K], F32, tag="et")
                nc.vector.tensor_scalar_mul(et[:], e[:], NEG)
                nc.vector.tensor_tensor(e[:], e[:], et[:], op=ALU.max)
                nc.scalar.activation(e[:], e[:], AF.Exp)

                dn = wk.tile([P, G], F32, tag="dn")
                nc.vector.tensor_reduce(dn[:], e[:], axis=AX.X, op=ALU.add)
                nc.vector.tensor_scalar_add(dn[:], dn[:], 1e-16)
                inv = wk.tile([P, G], F32, tag="inv")
                nc.vector.reciprocal(inv[:], dn[:])

                msg = wk.tile([P, G, D2, K], F32, tag="msg")
                nc.vector.tensor_tensor(
                    msg[:],
                    evt[:, :, 0 : 16 * K].rearrange("p g (d k) -> p g d k", k=K),
                    _mid0(e[:], 2, D2),
                    op=ALU.mult,
                )
                o = wk.tile([P, G, D2], F32, tag="o")
                nc.vector.tensor_reduce(o[:], msg[:], axis=AX.X, op=ALU.add)
                nc.vector.tensor_tensor(o[:], o[:], _tail0(inv[:], D2), op=ALU.mult)
                nc.vector.tensor_tensor(
                    o[:], o[:], _bc(b2r[:], [P, G, D2]), op=ALU.add
                )

                # log_softmax over the 16 classes
                nm = wk.tile([P, G], F32, tag="nm")
                nc.vector.tensor_reduce(nm[:], o[:], axis=AX.X, op=ALU.max, negate=True)
                nc.vector.tensor_tensor(o[:], o[:], _tail0(nm[:], D2), op=ALU.add)
                exq = wk.tile([P, G, D2], F32, tag="exq")
                nc.scalar.activation(exq[:], o[:], AF.Exp)
                ss = wk.tile([P, G], F32, tag="ss")
                nc.vector.tensor_reduce(ss[:], exq[:], axis=AX.X, op=ALU.add)
                nc.scalar.activation(ss[:], ss[:], AF.Ln)
                nc.vector.tensor_tensor(o[:], o[:], _tail0(ss[:], D2), op=ALU.subtract)

                nc.sync.dma_start(
                    out[t0 * P : (t0 + G) * P, :].rearrange("(g p) f -> p g f", p=P),
                    o[:],
                )
    nc.compile()
    return nc


# --------------------------------------------------------------------------
# Host orchestration
# --------------------------------------------------------------------------
def _make_groups(k_step, gmax, slot_budget):
    """Greedy: grow the group while tiles*K stays under slot_budget."""
    groups = []
    t0 = 0
    while t0 < STEPS:
        g = 1
        kb = max(int(k_step[t0]), 1)
        while (
            t0 + g < STEPS
            and g < gmax
            and (g + 1) * max(kb, int(k_step[t0 + g])) <= slot_budget
        ):
            kb = max(kb, int(k_step[t0 + g]))
            g += 1
        groups.append((t0, g, kb))
        t0 += g
    return groups


def _build_slots(groups, spos_node, deg, estart, src_by_dst):
    """slot -> src node id (N = pad) per core; layout per group is p-major:
    slot = base + p*(G*K) + g*K + k."""
    tot = sum(P * g * kb for _, g, kb in groups)
    slot = np.full((NC, tot), N, dtype=np.int64)
    arangeP = np.arange(P)
    for c in range(NC):
        base = 0
        for t0, g, kb in groups:
            for gi in range(g):
                T = (t0 + gi) * NC + c
                nodes = spos_node[T * P : (T + 1) * P]
                valid = nodes >= 0
                nv = nodes[valid]
                if nv.size == 0:
                    continue
                d = deg[nv]
                rowstart = base + arangeP[valid] * (g * kb) + gi * kb
                totd = int(d.sum())
                if totd == 0:
                    continue
                rep_row = np.repeat(rowstart, d)
                rep_cum = np.repeat(np.cumsum(d) - d, d)
                intra = np.arange(totd) - rep_cum
                rep_est = np.repeat(estart[nv], d)
                slot[c, rep_row + intra] = src_by_dst[rep_est + intra]
            base += P * g * kb
    return slot


def kernel(x, edge_index, W1, att_src1, att_dst1, b1, W2, att_src2, att_dst2, b2):
    x = np.asarray(x, dtype=np.float32)
    edge_index = np.asarray(edge_index)
    W1 = np.asarray(W1, dtype=np.float32)
    att_src1 = np.asarray(att_src1, dtype=np.float32)
    att_dst1 = np.asarray(att_dst1, dtype=np.float32)
    b1 = np.asarray(b1, dtype=np.float32)
    W2 = np.asarray(W2, dtype=np.float32)
    att_src2 = np.asarray(att_src2, dtype=np.float32).reshape(1, D2)
    att_dst2 = np.asarray(att_dst2, dtype=np.float32).reshape(1, D2)
    b2 = np.asarray(b2, dtype=np.float32)

    src = edge_index[0].astype(np.int64)
    dst = edge_index[1].astype(np.int64)

    # ---- schedule: degree-sorted tiles, round-robin dealt across cores ----
    deg = np.bincount(dst, minlength=N)
    order = np.argsort(deg, kind="stable")          # sorted-node space -> node id
    eo = np.argsort(dst, kind="stable")             # edges sorted by dst
    src_by_dst = src[eo]
    estart = np.zeros(N + 1, dtype=np.int64)
    estart[1:] = np.cumsum(deg)

    spos_node = np.full(TILES * P, -1, dtype=np.int64)
    spos_node[:N] = order
    sdeg = np.zeros(TILES * P, dtype=np.int64)
    sdeg[:N] = deg[order]
    tile_max = sdeg.reshape(TILES, P).max(axis=1)
    k_step = np.maximum(tile_max.reshape(STEPS, NC).max(axis=1), 1)  # [STEPS]

    groups2 = _make_groups(k_step, 4, 96)
    groups3 = _make_groups(k_step, 8, 200)
    slots2 = _build_slots(groups2, spos_node, deg, estart, src_by_dst)
    slots3 = _build_slots(groups3, spos_node, deg, estart, src_by_dst)
    ad_rows = np.where(spos_node < 0, N, spos_node)  # [TILES*P] node per row
    # per-core view: row t*128+p of core c <-> sorted pos (t*NC+c)*128+p
    ad_rows = (
        ad_rows.reshape(STEPS, NC, P).transpose(1, 0, 2).reshape(NC, NPC)
    )

    # ---- K1: node tables ----
    xpad = np.zeros((NC * NPC, F_IN), dtype=np.float32)
    xpad[:N] = x
    nc1 = build_k1()
    in1 = [
        {
            "xT": np.ascontiguousarray(xpad[c * NPC : (c + 1) * NPC].T),
            "w1": W1,
            "as1": att_src1,
            "ad1": att_dst1,
        }
        for c in range(NC)
    ]
    r1 = _run(nc1, in1, "k1")
    xq1 = np.empty((NC * NPC + 1, 80), dtype=np.float32)
    for c in range(NC):
        xq1[c * NPC : (c + 1) * NPC] = r1[c]["xq1T"].T
    xq1[-1] = 0.0
    xq1[-1, 64:72] = PADS                           # pad row: s1 = -1e38

    # ---- K2: layer 1 ----
    nc2 = build_k2(groups2, k_step)
    pad2 = np.where(slots2 >= N, NC * NPC, slots2)

    def _soa1(c):
        """Two streams, per (group, p, g) blocks, k innermost:
        s1 (8,K) and xp1 (8,8,K)."""
        rows = xq1[pad2[c], 0:72]
        outs = np.empty(rows.shape[0] * 8, dtype=np.float32)
        outx = np.empty(rows.shape[0] * 64, dtype=np.float32)
        bs = 0
        for _t0, g, kb in groups2:
            n = P * g * kb
            arr = rows[bs : bs + n].reshape(P, g, kb, 72)
            outs[bs * 8 : (bs + n) * 8] = (
                arr[..., 64:72].transpose(0, 1, 3, 2).ravel()
            )
            outx[bs * 64 : (bs + n) * 64] = (
                arr[..., 0:64].reshape(P, g, kb, 8, 8).transpose(0, 1, 3, 4, 2).ravel()
            )
            bs += n
        return outs, outx

    soa1 = [_soa1(c) for c in range(NC)]
    in2 = [
        {
            "ev1s": soa1[c][0],
            "ev1x": soa1[c][1],
            "adR": xq1[np.where(ad_rows[c] >= N, NC * NPC, ad_rows[c]), 72:80],
            "w2": W2,
            "as2": att_src2,
            "ad2": att_dst2,
            "b1": b1,
        }
        for c in range(NC)
    ]
    r2 = _run(nc2, in2, "k2")

    # reassemble layer-2 node table in original-node space
    t2 = np.zeros((N + 1, 18), dtype=np.float32)
    t2[N, 16] = PADS                                # pad row: s2 = -1e38
    for c in range(NC):
        cols = r2[c]["t2T"]                         # [18, NPC]
        rows = cols.T.reshape(STEPS, P, 18)
        for t in range(STEPS):
            T = t * NC + c
            nodes = spos_node[T * P : (T + 1) * P]
            valid = nodes >= 0
            t2[nodes[valid]] = rows[t][valid]

    # ---- K3: layer 2 ----
    nc3 = build_k3(groups3)
    pad3 = np.where(slots3 >= N, N, slots3)

    def _soa2(c):
        """Per (group, p, g) blocks: [xp2 (16,K) | s2 (K)], k innermost."""
        rows = t2[pad3[c]]
        out = np.empty(rows.shape[0] * 17, dtype=np.float32)
        bs = 0
        bf = 0
        for _t0, g, kb in groups3:
            n = P * g * kb
            arr = rows[bs : bs + n].reshape(P, g, kb, 18)
            xp = arr[..., 0:16].transpose(0, 1, 3, 2).reshape(P, g, 16 * kb)
            s = arr[..., 16].reshape(P, g, kb)
            out[bf : bf + n * 17] = np.concatenate([xp, s], axis=2).ravel()
            bs += n
            bf += n * 17
        return out

    in3 = [
        {
            "ev2": _soa2(c),
            "ad2R": t2[np.where(ad_rows[c] >= N, N, ad_rows[c]), 17:18],
            "b2": b2,
        }
        for c in range(NC)
    ]
    r3 = _run(nc3, in3, "k3")

    outp = np.zeros((N, D2), dtype=np.float32)
    for c in range(NC):
        o = r3[c]["o3"].reshape(STEPS, P, D2)
        for t in range(STEPS):
            T = t * NC + c
            nodes = spos_node[T * P : (T + 1) * P]
            valid = nodes >= 0
            outp[nodes[valid]] = o[t][valid]
    return outp



# revision 12
# speedup vs baseline: 1.2202x; 1.2202x over previous
"""GAT 2-layer network on 8 Trainium2 NeuronCores.

Strategy (edge-parallel, per the sharding hint "partition edges, replicate
node features"):
  - Nodes are sorted by in-degree and packed into 128-node tiles; tiles are
    dealt round-robin onto the 8 cores so every core runs the identical
    instruction stream (SPMD) over a shared per-step K schedule.
  - All FLOPs run on device across 4 launches:
      K1:  xq1b = [s1 | x@W1 + b1] (bf16) and xq1ad = ad1 (fp32) node tables.
      K2:  per dst-tile segment softmax + message aggregation for layer 1,
           then ELU -> h1 = elu(h)+1 node table (bf16).
      K2b: t2T = [xp2+b2-colsum | s2-colsum | ad2-colsum] = (h1-1)@W2ext + b2'
           via a matmul with a host-appended -1 ones-row (fp32 out).
      K3:  layer-2 segment softmax + aggregation + log_softmax.
  - Between launches the host only does index-based data movement: it
    replicates the device-computed per-node tables into per-edge-slot
    streams (degree-padded, p-major) so each device step reads purely
    sequential DMA.  Dtype casts fp32->bf16 are done with integer bit
    tricks (round-to-nearest-even); no floating-point math on the host.
  - Edge streams and message math run in bf16 (DVE 2x mode); softmax
    normalization and the log_softmax tail stay fp32.
"""

import os
import sys

for _p in ("/opt/trn_rl_repo", "/root/.axon_site/_ro/trn_rl_repo"):
    if os.path.isdir(_p) and _p not in sys.path:
        sys.path.insert(0, _p)

import numpy as np
import ml_dtypes

import concourse.bacc as bacc
import concourse.bass as bass
import concourse.tile as tile
from concourse import mybir
from concourse.bass_utils import run_bass_kernel_spmd

F32 = mybir.dt.float32
BF16 = mybir.dt.bfloat16
AF = mybir.ActivationFunctionType
ALU = mybir.AluOpType
AX = mybir.AxisListType
BF = ml_dtypes.bfloat16

N = 100000
E = 1600000
F_IN = 256
H1, D1 = 8, 8
HD1 = H1 * D1          # 64
D2 = 16                # H2 = 1
NEG = 0.2
NC = 8
P = 128
TILES = 784            # ceil(100000 / 128) rounded up to a multiple of 8
STEPS = TILES // NC    # 98
NPC = STEPS * P        # 12544 node rows handled per core
PADS = -1.0e38         # sentinel: exp(lrelu(PADS + ad)) == 0 exactly

PRELU = True           # use native parametric-relu on HW (CoreSim lacks it)
TRACE = False          # test.py flips this for NTFF profiling
SIM = False            # run through CoreSim instead of hardware
SIM_CORES = None       # e.g. [0] to only simulate core 0
LAST_EXEC_NS = []      # per-launch exec_time_ns when TRACE


def _run(nc, in_maps, tag):
    if SIM:
        from concourse.bass_interp import CoreSim

        outs = []
        cores = range(NC) if SIM_CORES is None else SIM_CORES
        for c in range(NC):
            if c not in cores:
                outs.append(outs[-1] if outs else {})
                continue
            sim = CoreSim(nc, trace=False)
            for k, v in in_maps[c].items():
                sim.tensor(k)[:] = v
            sim.simulate(check_with_hw=False)
            onames = [
                a.memorylocations[0].name
                for a in nc.m.functions[0].allocations
                if isinstance(a, mybir.MemoryLocationSet) and a.kind == "ExternalOutput"
            ]
            outs.append({k: np.array(sim.tensor(k)) for k in onames})
        return outs
    if TRACE:
        import hookfix  # noqa: F401  (registers antenv.axon_hooks)

        hookfix.install()
    res = run_bass_kernel_spmd(nc, in_maps, list(range(NC)), trace=TRACE)
    if TRACE:
        LAST_EXEC_NS.append((tag, res.exec_time_ns))
    return res.results


def _bc(ap, shape):
    """Broadcast the free dims of `ap` to `shape` (partition dim must already
    match).  Target dims are matched against source free dims right-to-left;
    size-1 source dims and unmatched target dims become step-0 (broadcast)."""
    src = ap.ap
    assert src[0][1] == shape[0], (src, shape)
    sdims = list(src[1:])
    res = []
    si = len(sdims) - 1
    for ti in range(len(shape) - 1, 0, -1):
        if si >= 0 and sdims[si][1] == shape[ti]:
            res.append(sdims[si])
            si -= 1
        elif si >= 0 and sdims[si][1] == 1:
            res.append([0, shape[ti]])
            si -= 1
        else:
            res.append([0, shape[ti]])
    assert si < 0, (src, shape)
    return bass.AP(tensor=ap.tensor, offset=ap.offset, ap=[src[0]] + res[::-1])


def _tail0(ap, n):
    """Append a trailing step-0 (broadcast) dim of size n."""
    return bass.AP(tensor=ap.tensor, offset=ap.offset, ap=list(ap.ap) + [[0, n]])


def _mid0(ap, pos, n):
    """Insert a step-0 (broadcast) dim of size n at free-dim position pos
    (ap.ap index pos, counting the partition dim as 0)."""
    dims = list(ap.ap)
    return bass.AP(
        tensor=ap.tensor, offset=ap.offset, ap=dims[:pos] + [[0, n]] + dims[pos:]
    )


def _rep_row(nc, pool, dram_t, nparts, cols, tag, dtype=F32):
    """DMA-replicate a flat `cols`-element DRAM tensor across `nparts`
    partitions (engines cannot broadcast across partitions themselves)."""
    tl = pool.tile([nparts, cols], dtype, tag=tag)
    src = bass.AP(tensor=dram_t[:].tensor, offset=0, ap=[[0, nparts], [1, cols]])
    nc.sync.dma_start(tl[:], src)
    return tl


def _to_bf16(a):
    """fp32 -> bf16 round-to-nearest-even via integer ops (no host FP math)."""
    a = np.ascontiguousarray(a, dtype=np.float32)
    u = a.view(np.uint32).astype(np.uint64)
    out = ((u + 0x7FFF + ((u >> 16) & 1)) >> 16).astype(np.uint16)
    return out.view(BF)


# --------------------------------------------------------------------------
# K1: node tables.  out xq1b [72, NPC] bf16: rows 0:64 xp1 = x@W1 + b1,
#     64:72 s1 (att_src dot).  xq1sa [16, NPC] fp32: rows 8:16 ad1.
# --------------------------------------------------------------------------
def build_k1():
    nc = bacc.Bacc("TRN2", target_bir_lowering=False, debug=False, num_devices=NC)
    xT = nc.dram_tensor("xT", [F_IN, NPC], BF16, kind="ExternalInput")
    w1 = nc.dram_tensor("w1", [F_IN, HD1], F32, kind="ExternalInput")
    as1 = nc.dram_tensor("as1", [H1, D1], F32, kind="ExternalInput")
    ad1 = nc.dram_tensor("ad1", [H1, D1], F32, kind="ExternalInput")
    b1t = nc.dram_tensor("b1", [HD1], F32, kind="ExternalInput")
    outb = nc.dram_tensor("xq1b", [72, NPC], BF16, kind="ExternalOutput")
    outa = nc.dram_tensor("xq1sa", [16, NPC], F32, kind="ExternalOutput")

    GT = 4                                      # node-tiles per matmul (PSUM bank)
    with tile.TileContext(nc) as tc:
        with (
            tc.tile_pool(name="pro", bufs=1) as pro,
            tc.tile_pool(name="io", bufs=3) as io,
            tc.tile_pool(name="ps", bufs=4, space="PSUM") as ps,
        ):
            w1sb = pro.tile([P, 2, HD1], F32)
            nc.sync.dma_start(w1sb[:], w1[:].rearrange("(c p) d -> p c d", p=P))
            asr = _rep_row(nc, pro, as1, P, HD1, "asr")
            adr = _rep_row(nc, pro, ad1, P, HD1, "adr")

            # wext cols: [0:64 W1 | 64:72 W1@att_src | 72:80 W1@att_dst]
            wext = pro.tile([P, 2, 80], F32)
            nc.scalar.copy(wext[:, :, 0:HD1], w1sb[:])
            for att, lo in ((asr, 64), (adr, 72)):
                tmp = pro.tile([P, 2, HD1], F32, tag="k1tmp")
                nc.vector.tensor_tensor(
                    tmp[:], w1sb[:], _bc(att[:], [P, 2, HD1]), op=ALU.mult
                )
                nc.vector.tensor_reduce(
                    wext[:, :, lo : lo + 8],
                    tmp[:].rearrange("p c (h d) -> p c h d", d=D1),
                    axis=AX.X,
                    op=ALU.add,
                )
            wextb = pro.tile([P, 2, 80], BF16)
            nc.vector.tensor_copy(wextb[:], wext[:])

            # bias rows for the bf16 copy: b1 ++ [0]*8
            b1e = pro.tile([72, 1], F32)
            nc.vector.memset(b1e[:], 0.0)
            nc.sync.dma_start(
                b1e[0:HD1, :],
                bass.AP(tensor=b1t[:].tensor, offset=0, ap=[[1, HD1], [1, 1]]),
            )

            xTr = xT[:].rearrange("(c p) n -> p c n", p=P)
            for t0 in range(0, STEPS, GT):
                g = min(GT, STEPS - t0)
                W = g * P
                xt = io.tile([P, 2, GT * P], BF16, tag="xt")
                nc.sync.dma_start(xt[:, :, 0:W], xTr[:, :, t0 * P : t0 * P + W])
                pt = ps.tile([80, GT * P], F32, tag="k1ps")
                nc.tensor.matmul(
                    pt[:, 0:W], lhsT=wextb[:, 0, :], rhs=xt[:, 0, 0:W],
                    start=True, stop=False,
                )
                nc.tensor.matmul(
                    pt[:, 0:W], lhsT=wextb[:, 1, :], rhs=xt[:, 1, 0:W],
                    start=False, stop=True,
                )
                ob = io.tile([72, GT * P], BF16, tag="k1ob")
                nc.scalar.activation(
                    ob[:, 0:W], pt[0:72, 0:W], AF.Identity, bias=b1e[:]
                )
                # PSUM partition offsets must be multiples of 32: read 64:80
                # (fp32); rows 0:8 of it duplicate s1, rows 8:16 are ad1.
                oa = io.tile([16, GT * P], F32, tag="k1oa")
                nc.vector.tensor_copy(oa[:, 0:W], pt[64:80, 0:W])
                nc.sync.dma_start(outb[:, t0 * P : t0 * P + W], ob[:, 0:W])
                nc.sync.dma_start(outa[:, t0 * P : t0 * P + W], oa[:, 0:W])
    nc.compile()
    return nc


# --------------------------------------------------------------------------
# K2: layer-1 edge aggregation + ELU(+1).
#   ev1 row (72 bf16): [xp1+b1 (64) | s1(8)] for the slot's src node (PADS
#   rows have s1 = -1e38 so exp()==0).  p-major slots: slot = base+p*GK+g*K+k.
#   Stream layout per (p, g): [72, K], k innermost.
#   out h1 [NPC, 64] bf16 = elu(agg/denom) + 1.
# --------------------------------------------------------------------------
def build_k2(groups):
    slots = P * sum(g * kb for _, g, kb in groups)
    nc = bacc.Bacc("TRN2", target_bir_lowering=False, debug=False, num_devices=NC)
    ev = nc.dram_tensor("ev1", [72 * slots], BF16, kind="ExternalInput")
    adt = nc.dram_tensor("adR", [NPC, H1], F32, kind="ExternalInput")
    out = nc.dram_tensor("h1", [NPC, HD1], BF16, kind="ExternalOutput")

    with tile.TileContext(nc) as tc:
        with (
            tc.tile_pool(name="io", bufs=2) as io,
            tc.tile_pool(name="wk", bufs=2) as wk,
        ):
            base = 0
            for t0, G, K in groups:
                evt = io.tile([P, G, 72, K], BF16, tag="ev")
                nc.sync.dma_start(
                    evt[:],
                    ev[72 * base : 72 * (base + P * G * K)].rearrange(
                        "(p g f k) -> p g f k", g=G, f=72, k=K
                    ),
                )
                base += P * G * K
                adv = io.tile([P, G, H1], F32, tag="ad")
                nc.sync.dma_start(
                    adv[:],
                    adt[t0 * P : (t0 + G) * P, :].rearrange("(g p) h -> p g h", p=P),
                )

                est = evt[:, :, 64:72, :]
                ext = evt[:, :, 0:64, :]

                # e = s1 + ad1 (gpsimd; leaves DVE free for the message ops)
                e = wk.tile([P, G, H1, K], BF16, tag="e")
                nc.gpsimd.tensor_tensor(e[:], est, _tail0(adv[:], K), op=ALU.add)

                # ex = exp(leaky_relu(e)), in place
                if PRELU:
                    nc.scalar.activation(e[:], e[:], AF.Prelu, alpha=NEG)
                    nc.scalar.activation(e[:], e[:], AF.Exp)
                    ex = e
                else:
                    r = wk.tile([P, G, H1, K], BF16, tag="r")
                    nc.scalar.activation(r[:], e[:], AF.Relu)
                    nc.scalar.activation(r[:], r[:], AF.Exp, scale=1.0 - NEG)
                    nc.scalar.activation(e[:], e[:], AF.Exp, scale=NEG)
                    ex = wk.tile([P, G, H1, K], BF16, tag="ex")
                    nc.vector.tensor_tensor(ex[:], e[:], r[:], op=ALU.mult)

                # denom (+eps) and reciprocal
                dn = wk.tile([P, G, H1], BF16, tag="dn")
                with nc.allow_low_precision(reason="single-round bf16 denom"):
                    nc.vector.tensor_reduce(dn[:], ex[:], axis=AX.X, op=ALU.add)
                    nc.vector.tensor_scalar_add(dn[:], dn[:], 1e-16)
                inv = wk.tile([P, G, H1], F32, tag="inv")
                nc.vector.reciprocal(inv[:], dn[:])

                # msg = ex * xp, in place in the stream tile, then reduce
                ex5 = _mid0(ex[:].rearrange("p g h k -> p g h k"), 3, D1)
                ext5 = ext.rearrange("p g (h d) k -> p g h d k", d=D1)
                nc.vector.tensor_tensor(ext5, ext5, ex5, op=ALU.mult)
                agg = wk.tile([P, G, H1, D1], BF16, tag="agg")
                with nc.allow_low_precision(reason="single-round bf16 agg"):
                    nc.vector.tensor_reduce(agg[:], ext5, axis=AX.X, op=ALU.add)

                # hraw = agg * inv; h1 = elu(hraw) + 1 = relu(hraw)+exp(min(,0))
                hraw = wk.tile([P, G, HD1], BF16, tag="hraw")
                nc.vector.tensor_tensor(
                    hraw[:].rearrange("p g (h d) -> p g h d", d=D1),
                    agg[:],
                    _tail0(inv[:], D1),
                    op=ALU.mult,
                )
                hpos = wk.tile([P, G, HD1], BF16, tag="hpos")
                nc.scalar.activation(hpos[:], hraw[:], AF.Relu)
                nc.vector.tensor_tensor(hraw[:], hraw[:], hpos[:], op=ALU.subtract)
                nc.scalar.activation(hraw[:], hraw[:], AF.Exp)  # exp(min(h,0))
                h1 = wk.tile([P, G, HD1], BF16, tag="h1")
                nc.vector.tensor_tensor(h1[:], hraw[:], hpos[:], op=ALU.add)

                nc.sync.dma_start(
                    out[t0 * P : (t0 + G) * P, :].rearrange("(g p) f -> p g f", p=P),
                    h1[:],
                )
    nc.compile()
    return nc


# --------------------------------------------------------------------------
# K2b: t2T = w2e_aug.T @ h1T_aug.  h1T_aug row 64 is -1 (host), w2e_aug row
#   64 is colsum(w2e) so the product computes (h1-1)@w2e = elu(h)@w2e.
#   b2 is folded into the xp2 columns via an extra bias row trick: instead we
#   add b2 on the K3 side (cheap), keeping this launch a pure matmul.
#   out t2T [18, NPC] fp32: rows 0:16 xp2, 16 s2, 17 ad2.
# --------------------------------------------------------------------------
def build_k2b():
    nc = bacc.Bacc("TRN2", target_bir_lowering=False, debug=False, num_devices=NC)
    h1T = nc.dram_tensor("h1T", [HD1 + 1, NPC], BF16, kind="ExternalInput")
    w2 = nc.dram_tensor("w2", [HD1, D2], F32, kind="ExternalInput")
    as2 = nc.dram_tensor("as2", [1, D2], F32, kind="ExternalInput")
    ad2 = nc.dram_tensor("ad2", [1, D2], F32, kind="ExternalInput")
    out = nc.dram_tensor("t2T", [18, NPC], F32, kind="ExternalOutput")

    W = 512
    with tile.TileContext(nc) as tc:
        with (
            tc.tile_pool(name="pro", bufs=1) as pro,
            tc.tile_pool(name="io", bufs=3) as io,
            tc.tile_pool(name="ps", bufs=4, space="PSUM") as ps,
        ):
            w2sb = pro.tile([HD1, D2], F32)
            nc.sync.dma_start(w2sb[:], w2[:])
            a2s = _rep_row(nc, pro, as2, HD1, D2, "a2s")
            a2d = _rep_row(nc, pro, ad2, HD1, D2, "a2d")

            # w2e [64, 18] = [W2 | W2@att_src2 | W2@att_dst2]
            w2e = pro.tile([HD1, 18], F32)
            nc.scalar.copy(w2e[:, 0:D2], w2sb[:])
            for att, col in ((a2s, 16), (a2d, 17)):
                tmp2 = pro.tile([HD1, D2], F32, tag="k2tmp")
                nc.vector.tensor_tensor(tmp2[:], w2sb[:], att[:], op=ALU.mult)
                nc.vector.tensor_reduce(
                    w2e[:, col : col + 1], tmp2[:], axis=AX.X, op=ALU.add
                )
            # augmented bf16 weights: row 64 = column sums (for the -1 shift)
            w2ab = pro.tile([HD1 + 1, 18], BF16)
            nc.vector.tensor_copy(w2ab[0:HD1, :], w2e[:])
            csum = pro.tile([1, 18], F32)
            nc.gpsimd.tensor_reduce(csum[:], w2e[:], axis=AX.C, op=ALU.add)
            nc.vector.tensor_copy(w2ab[HD1 : HD1 + 1, :], csum[:])

            for i, n0 in enumerate(range(0, NPC, W)):
                w = min(W, NPC - n0)
                ht = io.tile([HD1 + 1, W], BF16, tag="ht")
                nc.sync.dma_start(ht[:, 0:w], h1T[:, n0 : n0 + w])
                pt = ps.tile([18, W], F32, tag="pt")
                nc.tensor.matmul(
                    pt[:, 0:w], lhsT=w2ab[:], rhs=ht[:, 0:w], start=True, stop=True
                )
                st = io.tile([18, W], F32, tag="st")
                if i % 2 == 0:
                    nc.vector.tensor_copy(st[:, 0:w], pt[:, 0:w])
                else:
                    nc.scalar.copy(st[:, 0:w], pt[:, 0:w])
                nc.sync.dma_start(out[:, n0 : n0 + w], st[:, 0:w])
    nc.compile()
    return nc


# --------------------------------------------------------------------------
# K3: layer-2 edge aggregation + bias + log_softmax.
#   ev2 row (17 bf16): [xp2(16) | s2] for the slot's src node.
#   Stream layout per (p, g): [17, K], k innermost.
# --------------------------------------------------------------------------
def build_k3(groups):
    slots = P * sum(g * kb for _, g, kb in groups)
    nc = bacc.Bacc("TRN2", target_bir_lowering=False, debug=False, num_devices=NC)
    ev = nc.dram_tensor("ev2", [17 * slots], BF16, kind="ExternalInput")
    adt = nc.dram_tensor("ad2R", [NPC, 1], F32, kind="ExternalInput")
    b2t = nc.dram_tensor("b2", [D2], F32, kind="ExternalInput")
    out = nc.dram_tensor("o3", [NPC, D2], F32, kind="ExternalOutput")

    with tile.TileContext(nc) as tc:
        with (
            tc.tile_pool(name="pro", bufs=1) as pro,
            tc.tile_pool(name="io", bufs=2) as io,
            tc.tile_pool(name="wk", bufs=2) as wk,
        ):
            b2r = _rep_row(nc, pro, b2t, P, D2, "b2r")

            base = 0
            for t0, G, K in groups:
                evt = io.tile([P, G, 17, K], BF16, tag="ev")
                nc.sync.dma_start(
                    evt[:],
                    ev[17 * base : 17 * (base + P * G * K)].rearrange(
                        "(p g f k) -> p g f k", g=G, f=17, k=K
                    ),
                )
                base += P * G * K
                adv = io.tile([P, G, 1], F32, tag="ad")
                nc.sync.dma_start(
                    adv[:],
                    adt[t0 * P : (t0 + G) * P, :].rearrange("(g p) o -> p g o", p=P),
                )

                est = evt[:, :, 16, :]
                ext = evt[:, :, 0:16, :]

                e = wk.tile([P, G, K], BF16, tag="e")
                nc.gpsimd.tensor_tensor(
                    e[:], est, _bc(adv[:], [P, G, K]), op=ALU.add
                )
                if PRELU:
                    nc.scalar.activation(e[:], e[:], AF.Prelu, alpha=NEG)
                    nc.scalar.activation(e[:], e[:], AF.Exp)
                    ex = e
                else:
                    r = wk.tile([P, G, K], BF16, tag="r")
                    nc.scalar.activation(r[:], e[:], AF.Relu)
                    nc.scalar.activation(r[:], r[:], AF.Exp, scale=1.0 - NEG)
                    nc.scalar.activation(e[:], e[:], AF.Exp, scale=NEG)
                    ex = wk.tile([P, G, K], BF16, tag="ex")
                    nc.vector.tensor_tensor(ex[:], e[:], r[:], op=ALU.mult)

                dn = wk.tile([P, G], BF16, tag="dn")
                with nc.allow_low_precision(reason="single-round bf16 denom"):
                    nc.vector.tensor_reduce(dn[:], ex[:], axis=AX.X, op=ALU.add)
                    nc.vector.tensor_scalar_add(dn[:], dn[:], 1e-16)
                inv = wk.tile([P, G], F32, tag="inv")
                nc.vector.reciprocal(inv[:], dn[:])

                ex4 = _mid0(ex[:], 2, D2)
                nc.vector.tensor_tensor(ext, ext, ex4, op=ALU.mult)
                agg = wk.tile([P, G, D2], BF16, tag="agg")
                with nc.allow_low_precision(reason="single-round bf16 agg"):
                    nc.vector.tensor_reduce(agg[:], ext, axis=AX.X, op=ALU.add)

                # o = agg * inv + b2 (fp32 tail)
                o = wk.tile([P, G, D2], F32, tag="o")
                nc.vector.tensor_tensor(o[:], agg[:], _tail0(inv[:], D2), op=ALU.mult)
                nc.vector.tensor_tensor(
                    o[:], o[:], _bc(b2r[:], [P, G, D2]), op=ALU.add
                )

                # log_softmax over the 16 classes
                nm = wk.tile([P, G], F32, tag="nm")
                nc.vector.tensor_reduce(nm[:], o[:], axis=AX.X, op=ALU.max, negate=True)
                nc.vector.tensor_tensor(o[:], o[:], _tail0(nm[:], D2), op=ALU.add)
                exq = wk.tile([P, G, D2], F32, tag="exq")
                nc.scalar.activation(exq[:], o[:], AF.Exp)
                ss = wk.tile([P, G], F32, tag="ss")
                nc.vector.tensor_reduce(ss[:], exq[:], axis=AX.X, op=ALU.add)
                nc.scalar.activation(ss[:], ss[:], AF.Ln)
                nc.vector.tensor_tensor(o[:], o[:], _tail0(ss[:], D2), op=ALU.subtract)

                nc.sync.dma_start(
                    out[t0 * P : (t0 + G) * P, :].rearrange("(g p) f -> p g f", p=P),
                    o[:],
                )
    nc.compile()
    return nc


# --------------------------------------------------------------------------
# Host orchestration
# --------------------------------------------------------------------------
def _make_groups(k_step, gmax, slot_budget):
    """Greedy: grow the group while tiles*K stays under slot_budget."""
    groups = []
    t0 = 0
    while t0 < STEPS:
        g = 1
        kb = max(int(k_step[t0]), 1)
        while (
            t0 + g < STEPS
            and g < gmax
            and (g + 1) * max(kb, int(k_step[t0 + g])) <= slot_budget
        ):
            kb = max(kb, int(k_step[t0 + g]))
            g += 1
        groups.append((t0, g, kb))
        t0 += g
    return groups


def _build_slots(groups, spos_node, deg, estart, src_by_dst):
    """slot -> src node id (N = pad) per core; layout per group is p-major:
    slot = base + p*(G*K) + g*K + k."""
    tot = sum(P * g * kb for _, g, kb in groups)
    slot = np.full((NC, tot), N, dtype=np.int64)
    arangeP = np.arange(P)
    for c in range(NC):
        base = 0
        for t0, g, kb in groups:
            for gi in range(g):
                T = (t0 + gi) * NC + c
                nodes = spos_node[T * P : (T + 1) * P]
                valid = nodes >= 0
                nv = nodes[valid]
                if nv.size == 0:
                    continue
                d = deg[nv]
                rowstart = base + arangeP[valid] * (g * kb) + gi * kb
                totd = int(d.sum())
                if totd == 0:
                    continue
                rep_row = np.repeat(rowstart, d)
                rep_cum = np.repeat(np.cumsum(d) - d, d)
                intra = np.arange(totd) - rep_cum
                rep_est = np.repeat(estart[nv], d)
                slot[c, rep_row + intra] = src_by_dst[rep_est + intra]
            base += P * g * kb
    return slot


def kernel(x, edge_index, W1, att_src1, att_dst1, b1, W2, att_src2, att_dst2, b2):
    x = np.asarray(x, dtype=np.float32)
    edge_index = np.asarray(edge_index)
    W1 = np.asarray(W1, dtype=np.float32)
    att_src1 = np.asarray(att_src1, dtype=np.float32)
    att_dst1 = np.asarray(att_dst1, dtype=np.float32)
    b1 = np.asarray(b1, dtype=np.float32)
    W2 = np.asarray(W2, dtype=np.float32)
    att_src2 = np.asarray(att_src2, dtype=np.float32).reshape(1, D2)
    att_dst2 = np.asarray(att_dst2, dtype=np.float32).reshape(1, D2)
    b2 = np.asarray(b2, dtype=np.float32)

    src = edge_index[0].astype(np.int64)
    dst = edge_index[1].astype(np.int64)

    # ---- schedule: degree-sorted tiles, round-robin dealt across cores ----
    deg = np.bincount(dst, minlength=N)
    order = np.argsort(deg, kind="stable")          # sorted-node space -> node id
    eo = np.argsort(dst, kind="stable")             # edges sorted by dst
    src_by_dst = src[eo]
    estart = np.zeros(N + 1, dtype=np.int64)
    estart[1:] = np.cumsum(deg)

    spos_node = np.full(TILES * P, -1, dtype=np.int64)
    spos_node[:N] = order
    sdeg = np.zeros(TILES * P, dtype=np.int64)
    sdeg[:N] = deg[order]
    tile_max = sdeg.reshape(TILES, P).max(axis=1)
    k_step = np.maximum(tile_max.reshape(STEPS, NC).max(axis=1), 1)  # [STEPS]

    groups2 = _make_groups(k_step, 8, 300)
    groups3 = _make_groups(k_step, 16, 400)
    slots2 = _build_slots(groups2, spos_node, deg, estart, src_by_dst)
    slots3 = _build_slots(groups3, spos_node, deg, estart, src_by_dst)
    ad_rows = np.where(spos_node < 0, N, spos_node)  # [TILES*P] node per row
    # per-core view: row t*128+p of core c <-> sorted pos (t*NC+c)*128+p
    ad_rows = (
        ad_rows.reshape(STEPS, NC, P).transpose(1, 0, 2).reshape(NC, NPC)
    )

    # ---- K1: node tables ----
    xpad = np.zeros((NC * NPC, F_IN), dtype=np.float32)
    xpad[:N] = x
    xpad_b = _to_bf16(xpad)
    nc1 = build_k1()
    in1 = [
        {
            "xT": np.ascontiguousarray(xpad_b[c * NPC : (c + 1) * NPC].T),
            "w1": W1,
            "as1": att_src1,
            "ad1": att_dst1,
            "b1": b1,
        }
        for c in range(NC)
    ]
    r1 = _run(nc1, in1, "k1")
    # xq1b table [NC*NPC+1, 72] bf16 (stream-row format), xq1ad [NC*NPC+1, 8]
    xq1b = np.empty((NC * NPC + 1, 72), dtype=BF)
    xq1ad = np.empty((NC * NPC + 1, 8), dtype=np.float32)
    for c in range(NC):
        xq1b[c * NPC : (c + 1) * NPC] = np.asarray(r1[c]["xq1b"]).T
        xq1ad[c * NPC : (c + 1) * NPC] = np.asarray(r1[c]["xq1sa"]).T[:, 8:16]
    pad_row = np.zeros(72, dtype=BF)
    pad_row[64:72] = np.full(8, PADS, dtype=np.float32).astype(BF)
    xq1b[-1] = pad_row
    xq1ad[-1] = 0.0

    # ---- K2: layer 1 ----
    nc2 = build_k2(groups2)
    pad2 = np.where(slots2 >= N, NC * NPC, slots2)

    def _soa1(c):
        """Stream per (group, p, g) block: [72, K] rows (k innermost)."""
        rows = xq1b[pad2[c]]                        # [tot, 72] bf16
        outv = np.empty(rows.shape[0] * 72, dtype=BF)
        bs = 0
        for _t0, g, kb in groups2:
            n = P * g * kb
            arr = rows[bs : bs + n].reshape(P, g, kb, 72)
            outv[bs * 72 : (bs + n) * 72] = arr.transpose(0, 1, 3, 2).ravel()
            bs += n
        return outv

    in2 = [
        {
            "ev1": _soa1(c),
            "adR": xq1ad[np.where(ad_rows[c] >= N, NC * NPC, ad_rows[c])],
        }
        for c in range(NC)
    ]
    r2 = _run(nc2, in2, "k2")

    # ---- K2b: project h1 -> t2T ----
    nc2b = build_k2b()
    neg1 = np.full((1, NPC), -1.0, dtype=np.float32).astype(BF)
    in2b = [
        {
            "h1T": np.ascontiguousarray(
                np.vstack([np.asarray(r2[c]["h1"]).T, neg1])
            ),
            "w2": W2,
            "as2": att_src2,
            "ad2": att_dst2,
        }
        for c in range(NC)
    ]
    r2b = _run(nc2b, in2b, "k2b")

    # reassemble layer-2 node table in original-node space (bf16 stream rows)
    t2b = np.zeros((N + 1, 17), dtype=BF)
    t2b[N, 16] = np.float32(PADS).astype(BF)        # pad row: s2 = -1e38
    t2ad = np.zeros((N + 1, 1), dtype=np.float32)
    for c in range(NC):
        cols = np.asarray(r2b[c]["t2T"])            # [18, NPC] fp32
        rows17 = _to_bf16(cols[0:17].T).reshape(STEPS, P, 17)
        rowsad = cols[17].astype(np.float32).reshape(STEPS, P, 1)
        for t in range(STEPS):
            T = t * NC + c
            nodes = spos_node[T * P : (T + 1) * P]
            valid = nodes >= 0
            t2b[nodes[valid]] = rows17[t][valid]
            t2ad[nodes[valid]] = rowsad[t][valid]

    # ---- K3: layer 2 ----
    nc3 = build_k3(groups3)
    pad3 = np.where(slots3 >= N, N, slots3)

    def _soa2(c):
        """Stream per (group, p, g) block: [17, K] rows (k innermost)."""
        rows = t2b[pad3[c]]
        outv = np.empty(rows.shape[0] * 17, dtype=BF)
        bs = 0
        for _t0, g, kb in groups3:
            n = P * g * kb
            arr = rows[bs : bs + n].reshape(P, g, kb, 17)
            outv[bs * 17 : (bs + n) * 17] = arr.transpose(0, 1, 3, 2).ravel()
            bs += n
        return outv

    in3 = [
        {
            "ev2": _soa2(c),
            "ad2R": t2ad[np.where(ad_rows[c] >= N, N, ad_rows[c]), :],
            "b2": b2,
        }
        for c in range(NC)
    ]
    r3 = _run(nc3, in3, "k3")

    outp = np.zeros((N, D2), dtype=np.float32)
    for c in range(NC):
        o = np.asarray(r3[c]["o3"]).reshape(STEPS, P, D2)
        for t in range(STEPS):
            T = t * NC + c
            nodes = spos_node[T * P : (T + 1) * P]
            valid = nodes >= 0
            outp[nodes[valid]] = o[t][valid]
    return outp


# revision 22
# speedup vs baseline: 1.2869x; 1.0546x over previous
"""GAT 2-layer network on 8 Trainium2 NeuronCores.

Strategy (edge-parallel, per the sharding hint "partition edges, replicate
node features"):
  - Nodes are sorted by in-degree and packed into 128-node tiles; tiles are
    dealt round-robin onto the 8 cores so every core runs the identical
    instruction stream (SPMD) over a shared per-step K schedule.
  - All FLOPs run on device across 4 launches:
      K1:  xq1b = [s1 | x@W1 + b1] (bf16) and xq1ad = ad1 (fp32) node tables.
      K2:  per dst-tile segment softmax + message aggregation for layer 1,
           then ELU -> h1 = elu(h)+1 node table (bf16).
      K2b: t2T = [xp2+b2-colsum | s2-colsum | ad2-colsum] = (h1-1)@W2ext + b2'
           via a matmul with a host-appended -1 ones-row (fp32 out).
      K3:  layer-2 segment softmax + aggregation + log_softmax.
  - Between launches the host only does index-based data movement: it
    replicates the device-computed per-node tables into per-edge-slot
    streams (degree-padded, p-major) so each device step reads purely
    sequential DMA.  Dtype casts fp32->bf16 are done with integer bit
    tricks (round-to-nearest-even); no floating-point math on the host.
  - Edge streams and message math run in bf16 (DVE 2x mode); softmax
    normalization and the log_softmax tail stay fp32.
"""

import os
import sys

for _p in ("/opt/trn_rl_repo", "/root/.axon_site/_ro/trn_rl_repo"):
    if os.path.isdir(_p) and _p not in sys.path:
        sys.path.insert(0, _p)

import numpy as np
import ml_dtypes

import concourse.bacc as bacc
import concourse.bass as bass
import concourse.tile as tile
from concourse import mybir
from concourse.bass_utils import run_bass_kernel_spmd

F32 = mybir.dt.float32
BF16 = mybir.dt.bfloat16
AF = mybir.ActivationFunctionType
ALU = mybir.AluOpType
AX = mybir.AxisListType
BF = ml_dtypes.bfloat16

N = 100000
E = 1600000
F_IN = 256
H1, D1 = 8, 8
HD1 = H1 * D1          # 64
D2 = 16                # H2 = 1
NEG = 0.2
NC = 8
P = 128
TILES = 784            # ceil(100000 / 128) rounded up to a multiple of 8
STEPS = TILES // NC    # 98
NPC = STEPS * P        # 12544 node rows handled per core
PADS = -1.0e38         # sentinel: exp(lrelu(PADS + ad)) == 0 exactly

PRELU = True           # use native parametric-relu on HW (CoreSim lacks it)
TRACE = False          # test.py flips this for NTFF profiling
SIM = False            # run through CoreSim instead of hardware
SIM_CORES = None       # e.g. [0] to only simulate core 0
LAST_EXEC_NS = []      # per-launch exec_time_ns when TRACE


def _run(nc, in_maps, tag):
    if SIM:
        from concourse.bass_interp import CoreSim

        outs = []
        cores = range(NC) if SIM_CORES is None else SIM_CORES
        for c in range(NC):
            if c not in cores:
                outs.append(outs[-1] if outs else {})
                continue
            sim = CoreSim(nc, trace=False)
            for k, v in in_maps[c].items():
                sim.tensor(k)[:] = v
            sim.simulate(check_with_hw=False)
            onames = [
                a.memorylocations[0].name
                for a in nc.m.functions[0].allocations
                if isinstance(a, mybir.MemoryLocationSet) and a.kind == "ExternalOutput"
            ]
            outs.append({k: np.array(sim.tensor(k)) for k in onames})
        return outs
    if TRACE:
        import hookfix  # noqa: F401  (registers antenv.axon_hooks)

        hookfix.install()
    res = run_bass_kernel_spmd(nc, in_maps, list(range(NC)), trace=TRACE)
    if TRACE:
        LAST_EXEC_NS.append((tag, res.exec_time_ns))
    return res.results


def _bc(ap, shape):
    """Broadcast the free dims of `ap` to `shape` (partition dim must already
    match).  Target dims are matched against source free dims right-to-left;
    size-1 source dims and unmatched target dims become step-0 (broadcast)."""
    src = ap.ap
    assert src[0][1] == shape[0], (src, shape)
    sdims = list(src[1:])
    res = []
    si = len(sdims) - 1
    for ti in range(len(shape) - 1, 0, -1):
        if si >= 0 and sdims[si][1] == shape[ti]:
            res.append(sdims[si])
            si -= 1
        elif si >= 0 and sdims[si][1] == 1:
            res.append([0, shape[ti]])
            si -= 1
        else:
            res.append([0, shape[ti]])
    assert si < 0, (src, shape)
    return bass.AP(tensor=ap.tensor, offset=ap.offset, ap=[src[0]] + res[::-1])


def _tail0(ap, n):
    """Append a trailing step-0 (broadcast) dim of size n."""
    return bass.AP(tensor=ap.tensor, offset=ap.offset, ap=list(ap.ap) + [[0, n]])


def _mid0(ap, pos, n):
    """Insert a step-0 (broadcast) dim of size n at free-dim position pos
    (ap.ap index pos, counting the partition dim as 0)."""
    dims = list(ap.ap)
    return bass.AP(
        tensor=ap.tensor, offset=ap.offset, ap=dims[:pos] + [[0, n]] + dims[pos:]
    )


def _rep_row(nc, pool, dram_t, nparts, cols, tag, dtype=F32):
    """DMA-replicate a flat `cols`-element DRAM tensor across `nparts`
    partitions (engines cannot broadcast across partitions themselves)."""
    tl = pool.tile([nparts, cols], dtype, tag=tag)
    src = bass.AP(tensor=dram_t[:].tensor, offset=0, ap=[[0, nparts], [1, cols]])
    nc.sync.dma_start(tl[:], src)
    return tl


def _to_bf16(a):
    """fp32 -> bf16 round-to-nearest-even via integer ops (no host FP math)."""
    a = np.ascontiguousarray(a, dtype=np.float32)
    u = a.view(np.uint32).astype(np.uint64)
    out = ((u + 0x7FFF + ((u >> 16) & 1)) >> 16).astype(np.uint16)
    return out.view(BF)


def _tree_sum_k(nc, sl, K):
    """Sum over the innermost k-range of a tile in place with binary-halving
    tensor_tensor adds (these hit the DVE 2x bf16 mode; TENSOR_REDUCE does
    not).  `sl(k0, k1)` returns the AP for [..., k0:k1].  Returns the
    remaining prefix length for a final small tensor_reduce."""
    cur = K
    while cur > 4 and cur % 2 == 0:
        h = cur // 2
        nc.vector.tensor_tensor(sl(0, h), sl(0, h), sl(h, cur), op=ALU.add)
        cur = h
    return cur


# --------------------------------------------------------------------------
# K1: node tables.  out xq1b [72, NPC] bf16: rows 0:64 xp1 = x@W1 + b1,
#     64:72 s1 (att_src dot).  xq1sa [16, NPC] fp32: rows 8:16 ad1.
# --------------------------------------------------------------------------
def build_k1():
    nc = bacc.Bacc("TRN2", target_bir_lowering=False, debug=False, num_devices=NC)
    xT = nc.dram_tensor("xT", [F_IN, NPC], BF16, kind="ExternalInput")
    w1 = nc.dram_tensor("w1", [F_IN, HD1], F32, kind="ExternalInput")
    as1 = nc.dram_tensor("as1", [H1, D1], F32, kind="ExternalInput")
    ad1 = nc.dram_tensor("ad1", [H1, D1], F32, kind="ExternalInput")
    b1t = nc.dram_tensor("b1", [HD1], F32, kind="ExternalInput")
    outb = nc.dram_tensor("xq1b", [72, NPC], BF16, kind="ExternalOutput")
    outa = nc.dram_tensor("xq1sa", [16, NPC], F32, kind="ExternalOutput")

    GT = 16                                     # node-tiles per iteration
    MM = 4                                      # matmul column chunk = 1 bank
    with tile.TileContext(nc) as tc:
        with (
            tc.tile_pool(name="pro", bufs=1) as pro,
            tc.tile_pool(name="io", bufs=3) as io,
            tc.tile_pool(name="ps", bufs=2, space="PSUM") as ps,
        ):
            w1sb = pro.tile([P, 2, HD1], F32)
            nc.sync.dma_start(w1sb[:], w1[:].rearrange("(c p) d -> p c d", p=P))
            asr = _rep_row(nc, pro, as1, P, HD1, "asr")
            adr = _rep_row(nc, pro, ad1, P, HD1, "adr")

            # wext cols: [0:64 W1 | 64:72 W1@att_src | 72:80 W1@att_dst]
            wext = pro.tile([P, 2, 80], F32)
            nc.scalar.copy(wext[:, :, 0:HD1], w1sb[:])
            for att, lo in ((asr, 64), (adr, 72)):
                tmp = pro.tile([P, 2, HD1], F32, tag="k1tmp")
                nc.vector.tensor_tensor(
                    tmp[:], w1sb[:], _bc(att[:], [P, 2, HD1]), op=ALU.mult
                )
                nc.vector.tensor_reduce(
                    wext[:, :, lo : lo + 8],
                    tmp[:].rearrange("p c (h d) -> p c h d", d=D1),
                    axis=AX.X,
                    op=ALU.add,
                )
            wextb = pro.tile([P, 2, 80], BF16)
            nc.vector.tensor_copy(wextb[:], wext[:])

            # bias rows for the bf16 copy: b1 ++ [0]*8
            b1e = pro.tile([72, 1], F32)
            nc.vector.memset(b1e[:], 0.0)
            nc.sync.dma_start(
                b1e[0:HD1, :],
                bass.AP(tensor=b1t[:].tensor, offset=0, ap=[[1, HD1], [1, 1]]),
            )

            xTr = xT[:].rearrange("(c p) n -> p c n", p=P)
            for t0 in range(0, STEPS, GT):
                g = min(GT, STEPS - t0)
                W = g * P
                xt = io.tile([P, 2, GT * P], BF16, tag="xt")
                nc.sync.dma_start(xt[:, :, 0:W], xTr[:, :, t0 * P : t0 * P + W])
                pt = ps.tile([80, GT // MM, MM * P], F32, tag="k1ps")
                for j in range(0, g, MM):                   # 512-col PSUM banks
                    wj = min(MM, g - j) * P
                    for c in range(2):
                        nc.tensor.matmul(
                            pt[:, j // MM, 0:wj],
                            lhsT=wextb[:, c, :],
                            rhs=xt[:, c, j * P : j * P + wj],
                            start=(c == 0),
                            stop=(c == 1),
                        )
                ptv = pt[:].rearrange("r m w -> r (m w)")
                ob = io.tile([72, GT * P], BF16, tag="k1ob")
                nc.scalar.activation(
                    ob[:, 0:W], ptv[0:72, 0:W], AF.Identity, bias=b1e[:]
                )
                # PSUM partition offsets must be multiples of 32: read 64:80
                # (fp32); rows 0:8 of it duplicate s1, rows 8:16 are ad1.
                oa = io.tile([16, GT * P], F32, tag="k1oa")
                nc.vector.tensor_copy(oa[:, 0:W], ptv[64:80, 0:W])
                nc.sync.dma_start(outb[:, t0 * P : t0 * P + W], ob[:, 0:W])
                nc.sync.dma_start(outa[:, t0 * P : t0 * P + W], oa[:, 0:W])
    nc.compile()
    return nc


# --------------------------------------------------------------------------
# K2: layer-1 edge aggregation + ELU(+1).
#   ev1 row (72 bf16): [xp1+b1 (64) | s1(8)] for the slot's src node (PADS
#   rows have s1 = -1e38 so exp()==0).  p-major slots: slot = base+p*GK+g*K+k.
#   Stream layout per (p, g): [72, K], k innermost.
#   out h1 [NPC, 64] bf16 = elu(agg/denom) + 1.
# --------------------------------------------------------------------------
def build_k2(groups):
    slots = P * sum(g * kb for _, g, kb in groups)
    nc = bacc.Bacc("TRN2", target_bir_lowering=False, debug=False, num_devices=NC)
    ev = nc.dram_tensor("ev1", [72 * slots], BF16, kind="ExternalInput")
    adt = nc.dram_tensor("adR", [NPC, H1], F32, kind="ExternalInput")
    out = nc.dram_tensor("h1", [NPC, HD1], BF16, kind="ExternalOutput")

    with tile.TileContext(nc) as tc:
        with (
            tc.tile_pool(name="io", bufs=2) as io,
            tc.tile_pool(name="wk", bufs=2) as wk,
        ):
            base = 0
            for t0, G, K in groups:
                evt = io.tile([P, G, 72, K], BF16, tag="ev")
                nc.sync.dma_start(
                    evt[:],
                    ev[72 * base : 72 * (base + P * G * K)].rearrange(
                        "(p g f k) -> p g f k", g=G, f=72, k=K
                    ),
                )
                base += P * G * K
                adv = io.tile([P, G, H1], F32, tag="ad")
                nc.sync.dma_start(
                    adv[:],
                    adt[t0 * P : (t0 + G) * P, :].rearrange("(g p) h -> p g h", p=P),
                )

                est = evt[:, :, 64:72, :]
                ext = evt[:, :, 0:64, :]

                # e = s1 + ad1 (gpsimd; leaves DVE free for the message ops)
                e = wk.tile([P, G, H1, K], BF16, tag="e")
                nc.gpsimd.tensor_tensor(e[:], est, _tail0(adv[:], K), op=ALU.add)

                # ex = exp(leaky_relu(e)), in place
                if PRELU:
                    nc.scalar.activation(e[:], e[:], AF.Prelu, alpha=NEG)
                    nc.scalar.activation(e[:], e[:], AF.Exp)
                    ex = e
                else:
                    r = wk.tile([P, G, H1, K], BF16, tag="r")
                    nc.scalar.activation(r[:], e[:], AF.Relu)
                    nc.scalar.activation(r[:], r[:], AF.Exp, scale=1.0 - NEG)
                    nc.scalar.activation(e[:], e[:], AF.Exp, scale=NEG)
                    ex = wk.tile([P, G, H1, K], BF16, tag="ex")
                    nc.vector.tensor_tensor(ex[:], e[:], r[:], op=ALU.mult)

                # msg = ex * xp, in place in the stream tile
                ex5 = _mid0(ex[:], 3, D1)
                ext5 = ext.rearrange("p g (h d) k -> p g h d k", d=D1)
                nc.vector.tensor_tensor(ext5, ext5, ex5, op=ALU.mult)

                # denom: tree-halving sum over k in place on ex, then a small
                # reduce; +eps and reciprocal
                rk = _tree_sum_k(nc, lambda a, b: ex[:, :, :, a:b], K)
                dn = wk.tile([P, G, H1], BF16, tag="dn")
                with nc.allow_low_precision(reason="single-round bf16 denom"):
                    nc.vector.tensor_reduce(
                        dn[:], ex[:, :, :, 0:rk], axis=AX.X, op=ALU.add
                    )
                    nc.vector.tensor_scalar_add(dn[:], dn[:], 1e-16)
                inv = wk.tile([P, G, H1], F32, tag="inv")
                nc.vector.reciprocal(inv[:], dn[:])

                # agg: tree-halving sum over k in place on the msg tile
                rk = _tree_sum_k(nc, lambda a, b: evt[:, :, 0:64, a:b], K)
                agg = wk.tile([P, G, H1, D1], BF16, tag="agg")
                with nc.allow_low_precision(reason="single-round bf16 agg"):
                    nc.vector.tensor_reduce(
                        agg[:],
                        ext.rearrange("p g (h d) k -> p g h d k", d=D1)[
                            :, :, :, :, 0:rk
                        ],
                        axis=AX.X,
                        op=ALU.add,
                    )

                # hraw = agg * inv on gpsimd (DVE is the bottleneck engine)
                hraw = wk.tile([P, G, HD1], BF16, tag="hraw")
                nc.gpsimd.tensor_tensor(
                    hraw[:].rearrange("p g (h d) -> p g h d", d=D1),
                    agg[:],
                    _tail0(inv[:], D1),
                    op=ALU.mult,
                )
                hpos = wk.tile([P, G, HD1], BF16, tag="hpos")
                nc.scalar.activation(hpos[:], hraw[:], AF.Relu)
                nc.vector.tensor_tensor(hraw[:], hraw[:], hpos[:], op=ALU.subtract)
                nc.scalar.activation(hraw[:], hraw[:], AF.Exp)  # exp(min(h,0))
                h1 = wk.tile([P, G, HD1], BF16, tag="h1")
                nc.vector.tensor_tensor(h1[:], hraw[:], hpos[:], op=ALU.add)

                nc.sync.dma_start(
                    out[t0 * P : (t0 + G) * P, :].rearrange("(g p) f -> p g f", p=P),
                    h1[:],
                )
    nc.compile()
    return nc


# --------------------------------------------------------------------------
# K2b: t2T = w2e_aug.T @ h1T_aug.  h1T_aug row 64 is -1 (host), w2e_aug row
#   64 is colsum(w2e) so the product computes (h1-1)@w2e = elu(h)@w2e.
#   b2 is folded into the xp2 columns via an extra bias row trick: instead we
#   add b2 on the K3 side (cheap), keeping this launch a pure matmul.
#   out t2T [18, NPC] fp32: rows 0:16 xp2, 16 s2, 17 ad2.
# --------------------------------------------------------------------------
def build_k2b():
    nc = bacc.Bacc("TRN2", target_bir_lowering=False, debug=False, num_devices=NC)
    h1T = nc.dram_tensor("h1T", [HD1 + 1, NPC], BF16, kind="ExternalInput")
    w2 = nc.dram_tensor("w2", [HD1, D2], F32, kind="ExternalInput")
    as2 = nc.dram_tensor("as2", [1, D2], F32, kind="ExternalInput")
    ad2 = nc.dram_tensor("ad2", [1, D2], F32, kind="ExternalInput")
    out = nc.dram_tensor("t2T", [18, NPC], F32, kind="ExternalOutput")

    W = 2048
    MB = 512                                    # matmul chunk = 1 PSUM bank
    with tile.TileContext(nc) as tc:
        with (
            tc.tile_pool(name="pro", bufs=1) as pro,
            tc.tile_pool(name="io", bufs=3) as io,
            tc.tile_pool(name="ps", bufs=2, space="PSUM") as ps,
        ):
            w2sb = pro.tile([HD1, D2], F32)
            nc.sync.dma_start(w2sb[:], w2[:])
            a2s = _rep_row(nc, pro, as2, HD1, D2, "a2s")
            a2d = _rep_row(nc, pro, ad2, HD1, D2, "a2d")

            # w2e [64, 18] = [W2 | W2@att_src2 | W2@att_dst2]
            w2e = pro.tile([HD1, 18], F32)
            nc.scalar.copy(w2e[:, 0:D2], w2sb[:])
            for att, col in ((a2s, 16), (a2d, 17)):
                tmp2 = pro.tile([HD1, D2], F32, tag="k2tmp")
                nc.vector.tensor_tensor(tmp2[:], w2sb[:], att[:], op=ALU.mult)
                nc.vector.tensor_reduce(
                    w2e[:, col : col + 1], tmp2[:], axis=AX.X, op=ALU.add
                )
            # augmented bf16 weights: row 64 = column sums (for the -1 shift)
            w2ab = pro.tile([HD1 + 1, 18], BF16)
            nc.vector.tensor_copy(w2ab[0:HD1, :], w2e[:])
            csum = pro.tile([1, 18], F32)
            nc.gpsimd.tensor_reduce(csum[:], w2e[:], axis=AX.C, op=ALU.add)
            nc.vector.tensor_copy(w2ab[HD1 : HD1 + 1, :], csum[:])

            for i, n0 in enumerate(range(0, NPC, W)):
                w = min(W, NPC - n0)
                ht = io.tile([HD1 + 1, W], BF16, tag="ht")
                nc.sync.dma_start(ht[:, 0:w], h1T[:, n0 : n0 + w])
                pt = ps.tile([18, W // MB, MB], F32, tag="pt")
                for j in range(0, w, MB):
                    wj = min(MB, w - j)
                    nc.tensor.matmul(
                        pt[:, j // MB, 0:wj],
                        lhsT=w2ab[:],
                        rhs=ht[:, j : j + wj],
                        start=True,
                        stop=True,
                    )
                ptv = pt[:].rearrange("r m w -> r (m w)")
                st = io.tile([18, W], F32, tag="st")
                if i % 2 == 0:
                    nc.vector.tensor_copy(st[:, 0:w], ptv[:, 0:w])
                else:
                    nc.scalar.copy(st[:, 0:w], ptv[:, 0:w])
                nc.sync.dma_start(out[:, n0 : n0 + w], st[:, 0:w])
    nc.compile()
    return nc


# --------------------------------------------------------------------------
# K3: layer-2 edge aggregation + bias + log_softmax.
#   ev2 row (17 bf16): [xp2(16) | s2] for the slot's src node.
#   Stream layout per (p, g): [17, K], k innermost.
# --------------------------------------------------------------------------
def build_k3(groups):
    slots = P * sum(g * kb for _, g, kb in groups)
    nc = bacc.Bacc("TRN2", target_bir_lowering=False, debug=False, num_devices=NC)
    ev = nc.dram_tensor("ev2", [17 * slots], BF16, kind="ExternalInput")
    adt = nc.dram_tensor("ad2R", [NPC, 1], F32, kind="ExternalInput")
    b2t = nc.dram_tensor("b2", [D2], F32, kind="ExternalInput")
    out = nc.dram_tensor("o3", [NPC, D2], F32, kind="ExternalOutput")

    with tile.TileContext(nc) as tc:
        with (
            tc.tile_pool(name="pro", bufs=1) as pro,
            tc.tile_pool(name="io", bufs=2) as io,
            tc.tile_pool(name="wk", bufs=2) as wk,
        ):
            b2r = _rep_row(nc, pro, b2t, P, D2, "b2r")

            base = 0
            for t0, G, K in groups:
                evt = io.tile([P, G, 17, K], BF16, tag="ev")
                nc.sync.dma_start(
                    evt[:],
                    ev[17 * base : 17 * (base + P * G * K)].rearrange(
                        "(p g f k) -> p g f k", g=G, f=17, k=K
                    ),
                )
                base += P * G * K
                adv = io.tile([P, G, 1], F32, tag="ad")
                nc.sync.dma_start(
                    adv[:],
                    adt[t0 * P : (t0 + G) * P, :].rearrange("(g p) o -> p g o", p=P),
                )

                est = evt[:, :, 16, :]
                ext = evt[:, :, 0:16, :]

                e = wk.tile([P, G, K], BF16, tag="e")
                nc.gpsimd.tensor_tensor(
                    e[:], est, _bc(adv[:], [P, G, K]), op=ALU.add
                )
                if PRELU:
                    nc.scalar.activation(e[:], e[:], AF.Prelu, alpha=NEG)
                    nc.scalar.activation(e[:], e[:], AF.Exp)
                    ex = e
                else:
                    r = wk.tile([P, G, K], BF16, tag="r")
                    nc.scalar.activation(r[:], e[:], AF.Relu)
                    nc.scalar.activation(r[:], r[:], AF.Exp, scale=1.0 - NEG)
                    nc.scalar.activation(e[:], e[:], AF.Exp, scale=NEG)
                    ex = wk.tile([P, G, K], BF16, tag="ex")
                    nc.vector.tensor_tensor(ex[:], e[:], r[:], op=ALU.mult)

                ex4 = _mid0(ex[:], 2, D2)
                nc.vector.tensor_tensor(ext, ext, ex4, op=ALU.mult)

                rk = _tree_sum_k(nc, lambda a, b: ex[:, :, a:b], K)
                dn = wk.tile([P, G], BF16, tag="dn")
                with nc.allow_low_precision(reason="single-round bf16 denom"):
                    nc.vector.tensor_reduce(
                        dn[:], ex[:, :, 0:rk], axis=AX.X, op=ALU.add
                    )
                    nc.vector.tensor_scalar_add(dn[:], dn[:], 1e-16)
                inv = wk.tile([P, G], F32, tag="inv")
                nc.vector.reciprocal(inv[:], dn[:])

                rk = _tree_sum_k(nc, lambda a, b: evt[:, :, 0:16, a:b], K)
                agg = wk.tile([P, G, D2], BF16, tag="agg")
                with nc.allow_low_precision(reason="single-round bf16 agg"):
                    nc.vector.tensor_reduce(
                        agg[:], evt[:, :, 0:16, 0:rk], axis=AX.X, op=ALU.add
                    )

                # o = agg * inv + b2 (fp32 tail)
                o = wk.tile([P, G, D2], F32, tag="o")
                nc.vector.tensor_tensor(o[:], agg[:], _tail0(inv[:], D2), op=ALU.mult)
                nc.vector.tensor_tensor(
                    o[:], o[:], _bc(b2r[:], [P, G, D2]), op=ALU.add
                )

                # log_softmax over the 16 classes
                nm = wk.tile([P, G], F32, tag="nm")
                nc.vector.tensor_reduce(nm[:], o[:], axis=AX.X, op=ALU.max, negate=True)
                nc.vector.tensor_tensor(o[:], o[:], _tail0(nm[:], D2), op=ALU.add)
                exq = wk.tile([P, G, D2], F32, tag="exq")
                nc.scalar.activation(exq[:], o[:], AF.Exp)
                ss = wk.tile([P, G], F32, tag="ss")
                nc.vector.tensor_reduce(ss[:], exq[:], axis=AX.X, op=ALU.add)
                nc.scalar.activation(ss[:], ss[:], AF.Ln)
                nc.vector.tensor_tensor(o[:], o[:], _tail0(ss[:], D2), op=ALU.subtract)

                nc.sync.dma_start(
                    out[t0 * P : (t0 + G) * P, :].rearrange("(g p) f -> p g f", p=P),
                    o[:],
                )
    nc.compile()
    return nc


# --------------------------------------------------------------------------
# Host orchestration
# --------------------------------------------------------------------------
def _make_groups(k_step, gmax, slot_budget):
    """Greedy: grow the group while tiles*K stays under slot_budget.  K is
    rounded up to a multiple of 4 so the tree reduction halves cleanly."""

    def r4(k):
        return (k + 3) // 4 * 4

    groups = []
    t0 = 0
    while t0 < STEPS:
        g = 1
        kb = max(int(k_step[t0]), 1)
        while (
            t0 + g < STEPS
            and g < gmax
            and (g + 1) * r4(max(kb, int(k_step[t0 + g]))) <= slot_budget
        ):
            kb = max(kb, int(k_step[t0 + g]))
            g += 1
        groups.append((t0, g, r4(kb)))
        t0 += g
    return groups


def _build_slots(groups, spos_node, deg, estart, src_by_dst):
    """slot -> src node id (N = pad) per core; layout per group is p-major:
    slot = base + p*(G*K) + g*K + k."""
    tot = sum(P * g * kb for _, g, kb in groups)
    slot = np.full((NC, tot), N, dtype=np.int64)
    arangeP = np.arange(P)
    for c in range(NC):
        base = 0
        for t0, g, kb in groups:
            for gi in range(g):
                T = (t0 + gi) * NC + c
                nodes = spos_node[T * P : (T + 1) * P]
                valid = nodes >= 0
                nv = nodes[valid]
                if nv.size == 0:
                    continue
                d = deg[nv]
                rowstart = base + arangeP[valid] * (g * kb) + gi * kb
                totd = int(d.sum())
                if totd == 0:
                    continue
                rep_row = np.repeat(rowstart, d)
                rep_cum = np.repeat(np.cumsum(d) - d, d)
                intra = np.arange(totd) - rep_cum
                rep_est = np.repeat(estart[nv], d)
                slot[c, rep_row + intra] = src_by_dst[rep_est + intra]
            base += P * g * kb
    return slot


def kernel(x, edge_index, W1, att_src1, att_dst1, b1, W2, att_src2, att_dst2, b2):
    x = np.asarray(x, dtype=np.float32)
    edge_index = np.asarray(edge_index)
    W1 = np.asarray(W1, dtype=np.float32)
    att_src1 = np.asarray(att_src1, dtype=np.float32)
    att_dst1 = np.asarray(att_dst1, dtype=np.float32)
    b1 = np.asarray(b1, dtype=np.float32)
    W2 = np.asarray(W2, dtype=np.float32)
    att_src2 = np.asarray(att_src2, dtype=np.float32).reshape(1, D2)
    att_dst2 = np.asarray(att_dst2, dtype=np.float32).reshape(1, D2)
    b2 = np.asarray(b2, dtype=np.float32)

    src = edge_index[0].astype(np.int64)
    dst = edge_index[1].astype(np.int64)

    # ---- schedule: degree-sorted tiles, round-robin dealt across cores ----
    deg = np.bincount(dst, minlength=N)
    order = np.argsort(deg, kind="stable")          # sorted-node space -> node id
    eo = np.argsort(dst, kind="stable")             # edges sorted by dst
    src_by_dst = src[eo]
    estart = np.zeros(N + 1, dtype=np.int64)
    estart[1:] = np.cumsum(deg)

    spos_node = np.full(TILES * P, -1, dtype=np.int64)
    spos_node[:N] = order
    sdeg = np.zeros(TILES * P, dtype=np.int64)
    sdeg[:N] = deg[order]
    tile_max = sdeg.reshape(TILES, P).max(axis=1)
    k_step = np.maximum(tile_max.reshape(STEPS, NC).max(axis=1), 1)  # [STEPS]

    groups2 = _make_groups(k_step, 8, 320)
    groups3 = _make_groups(k_step, 16, 640)
    slots2 = _build_slots(groups2, spos_node, deg, estart, src_by_dst)
    slots3 = _build_slots(groups3, spos_node, deg, estart, src_by_dst)
    ad_rows = np.where(spos_node < 0, N, spos_node)  # [TILES*P] node per row
    # per-core view: row t*128+p of core c <-> sorted pos (t*NC+c)*128+p
    ad_rows = (
        ad_rows.reshape(STEPS, NC, P).transpose(1, 0, 2).reshape(NC, NPC)
    )

    # ---- K1: node tables ----
    xpad = np.zeros((NC * NPC, F_IN), dtype=np.float32)
    xpad[:N] = x
    xpad_b = _to_bf16(xpad)
    nc1 = build_k1()
    in1 = [
        {
            "xT": np.ascontiguousarray(xpad_b[c * NPC : (c + 1) * NPC].T),
            "w1": W1,
            "as1": att_src1,
            "ad1": att_dst1,
            "b1": b1,
        }
        for c in range(NC)
    ]
    r1 = _run(nc1, in1, "k1")
    # xq1b table [NC*NPC+1, 72] bf16 (stream-row format), xq1ad [NC*NPC+1, 8]
    xq1b = np.empty((NC * NPC + 1, 72), dtype=BF)
    xq1ad = np.empty((NC * NPC + 1, 8), dtype=np.float32)
    for c in range(NC):
        xq1b[c * NPC : (c + 1) * NPC] = np.asarray(r1[c]["xq1b"]).T
        xq1ad[c * NPC : (c + 1) * NPC] = np.asarray(r1[c]["xq1sa"]).T[:, 8:16]
    pad_row = np.zeros(72, dtype=BF)
    pad_row[64:72] = np.full(8, PADS, dtype=np.float32).astype(BF)
    xq1b[-1] = pad_row
    xq1ad[-1] = 0.0

    # ---- K2: layer 1 ----
    nc2 = build_k2(groups2)
    pad2 = np.where(slots2 >= N, NC * NPC, slots2)

    def _soa1(c):
        """Stream per (group, p, g) block: [72, K] rows (k innermost)."""
        rows = xq1b[pad2[c]]                        # [tot, 72] bf16
        outv = np.empty(rows.shape[0] * 72, dtype=BF)
        bs = 0
        for _t0, g, kb in groups2:
            n = P * g * kb
            arr = rows[bs : bs + n].reshape(P, g, kb, 72)
            outv[bs * 72 : (bs + n) * 72] = arr.transpose(0, 1, 3, 2).ravel()
            bs += n
        return outv

    in2 = [
        {
            "ev1": _soa1(c),
            "adR": xq1ad[np.where(ad_rows[c] >= N, NC * NPC, ad_rows[c])],
        }
        for c in range(NC)
    ]
    r2 = _run(nc2, in2, "k2")

    # ---- K2b: project h1 -> t2T ----
    nc2b = build_k2b()
    neg1 = np.full((1, NPC), -1.0, dtype=np.float32).astype(BF)
    in2b = [
        {
            "h1T": np.ascontiguousarray(
                np.vstack([np.asarray(r2[c]["h1"]).T, neg1])
            ),
            "w2": W2,
            "as2": att_src2,
            "ad2": att_dst2,
        }
        for c in range(NC)
    ]
    r2b = _run(nc2b, in2b, "k2b")

    # reassemble layer-2 node table in original-node space (bf16 stream rows)
    t2b = np.zeros((N + 1, 17), dtype=BF)
    t2b[N, 16] = np.float32(PADS).astype(BF)        # pad row: s2 = -1e38
    t2ad = np.zeros((N + 1, 1), dtype=np.float32)
    for c in range(NC):
        cols = np.asarray(r2b[c]["t2T"])            # [18, NPC] fp32
        rows17 = _to_bf16(cols[0:17].T).reshape(STEPS, P, 17)
        rowsad = cols[17].astype(np.float32).reshape(STEPS, P, 1)
        for t in range(STEPS):
            T = t * NC + c
            nodes = spos_node[T * P : (T + 1) * P]
            valid = nodes >= 0
            t2b[nodes[valid]] = rows17[t][valid]
            t2ad[nodes[valid]] = rowsad[t][valid]

    # ---- K3: layer 2 ----
    nc3 = build_k3(groups3)
    pad3 = np.where(slots3 >= N, N, slots3)

    def _soa2(c):
        """Stream per (group, p, g) block: [17, K] rows (k innermost)."""
        rows = t2b[pad3[c]]
        outv = np.empty(rows.shape[0] * 17, dtype=BF)
        bs = 0
        for _t0, g, kb in groups3:
            n = P * g * kb
            arr = rows[bs : bs + n].reshape(P, g, kb, 17)
            outv[bs * 17 : (bs + n) * 17] = arr.transpose(0, 1, 3, 2).ravel()
            bs += n
        return outv

    in3 = [
        {
            "ev2": _soa2(c),
            "ad2R": t2ad[np.where(ad_rows[c] >= N, N, ad_rows[c]), :],
            "b2": b2,
        }
        for c in range(NC)
    ]
    r3 = _run(nc3, in3, "k3")

    outp = np.zeros((N, D2), dtype=np.float32)
    for c in range(NC):
        o = np.asarray(r3[c]["o3"]).reshape(STEPS, P, D2)
        for t in range(STEPS):
            T = t * NC + c
            nodes = spos_node[T * P : (T + 1) * P]
            valid = nodes >= 0
            outp[nodes[valid]] = o[t][valid]
    return outp
